# revision 28
# baseline (speedup 1.0000x reference)
"""Trainium2 Bass kernel for DP MultiHeadAttention.

Problem: B=2, S=2048, D=1024, H=16, DH=64 (fp32).
  q/k/v = per-head projections of x; scores = q k^T / 8; probs = softmax;
  ctx = probs @ v; out = concat-heads(ctx) @ Wo + bo.

Sharding: heads (tensor parallel) across 8 cores, 2 heads/core.
Each core computes its 2 heads' q/k/v + attention + the partial output
projection (its 128 rows of Wo); host sums the 8 partials + bo
(the "all-reduce" of the output projection, done at gather time).

Device algorithm per core (transposed-softmax flash-style layout):
  - Host supplies x pre-transposed: xT [B, D, S].
  - qT/kT [128=2*DH, S] = Wqkv^T-style projections (PE, f32r), bias added
    per-partition on DVE.
  - v produced transposed then PE-transposed back to natural [S, 2*DH],
    stored per sj-chunk as [128, 16, 130] with a ones-column per head
    (the ones column makes the ctx matmul also produce the softmax
    denominator for free).
  - scoresT[sj, si] per head via row-packed K=64 matmul pairs.
  - exp on ScalarE (scale=1/8 folded in, no max-subtraction: scores are
    O(5) so exp is safe in fp32).
  - ctx^T accumulated over sj on PE; row 64 = denominator.
  - recip on DVE, broadcast across partitions via K=1 PE outer product,
    ctx scaled on DVE -> ctxs [64, S] per head (f32r).
  - output projection: two K=64 accumulating matmuls (one per head)
    per [128, 512] output tile.

All matmuls use float32r (1 cycle/row vs 4 for strict fp32; ~1.5e-4 rel
err measured on HW) with fp32 PSUM accumulation.
"""
import os
import sys

for _p in ("/opt/trn_rl_repo",):
    if _p not in sys.path:
        sys.path.insert(0, _p)

import numpy as np

import concourse.bass as bass
from concourse import bacc, mybir, masks
from concourse import tile as ctile
from concourse import bass_utils
from contextlib import ExitStack

B, S, D, H = 2, 2048, 1024, 16
DH = D // H  # 64
NCORES = 8
HLOC = H // NCORES  # 2
SW = 512            # si window
NSW = S // SW       # 4
NSJ = S // 128      # 16
NDC = D // 128      # 8

F32 = mybir.dt.float32
F32R = mybir.dt.float32r
AF = mybir.ActivationFunctionType


def _build(reps: int = 1, loop_reps: int = 1):
    # The neuron NEFF cache keys can collide across different BIR contents
    # (observed during development); never trust a stale cache.
    import shutil
    shutil.rmtree(os.path.expanduser("~/.neuron-compile-cache"),
                  ignore_errors=True)
    nc = bacc.Bacc("TRN2", target_bir_lowering=False, debug=False,
                   num_devices=NCORES)
    xT_d = nc.dram_tensor("xT", [B, D, S], F32R, kind="ExternalInput")
    wqkv_d = nc.dram_tensor("wqkv", [D, 3 * 128], F32R, kind="ExternalInput")
    bias_d = nc.dram_tensor("bqkv", [128, 3], F32, kind="ExternalInput")
    wo_d = nc.dram_tensor("wo", [128, D], F32R, kind="ExternalInput")
    out_d = nc.dram_tensor("out", [B, S, D], F32, kind="ExternalOutput")

    with ctile.TileContext(nc) as tc, ExitStack() as ctx:
        const = ctx.enter_context(tc.tile_pool(name="const", bufs=1))
        xcp = ctx.enter_context(tc.tile_pool(name="xcp", bufs=10))
        qkp = ctx.enter_context(tc.tile_pool(name="qkp", bufs=2))
        vtp = ctx.enter_context(tc.tile_pool(name="vtp", bufs=2))
        ep = ctx.enter_context(tc.tile_pool(name="ep", bufs=10))
        ctxsp = ctx.enter_context(tc.tile_pool(name="ctxsp", bufs=2))
        smp = ctx.enter_context(tc.tile_pool(name="smp", bufs=2))
        osp = ctx.enter_context(tc.tile_pool(name="osp", bufs=3))
        sc_p = ctx.enter_context(
            tc.tile_pool(name="sc_p", bufs=2, space=bass.MemorySpace.PSUM))
        ctx_p = ctx.enter_context(
            tc.tile_pool(name="ctx_p", bufs=2, space=bass.MemorySpace.PSUM))
        gp_p = ctx.enter_context(
            tc.tile_pool(name="gp_p", bufs=2, space=bass.MemorySpace.PSUM))

        # ---- constants ----
        wqkv_sb = const.tile([128, NDC, 3 * 128], F32R)
        for dc in range(NDC):
            nc.sync.dma_start(wqkv_sb[:, dc, :],
                                wqkv_d.ap()[dc * 128:(dc + 1) * 128, :])
        bias_sb = const.tile([128, 3], F32)
        nc.sync.dma_start(bias_sb[:], bias_d.ap()[:])
        wo_sb = const.tile([128, D], F32R)
        nc.sync.dma_start(wo_sb[:], wo_d.ap()[:])
        ident = const.tile([128, 128], F32)
        masks.make_identity(nc, ident[:])
        # ones row (bcast matmul lhsT)
        ones_sb = const.tile([1, 64], F32R)
        nc.gpsimd.memset(ones_sb[:].bitcast(F32), 1.0)

        def _emit_body():
            for b in range(B):
                qT = qkp.tile([128, S], F32R, tag="qT")
                kT = qkp.tile([128, S], F32R, tag="kT")
                v_sb = vtp.tile([128, NSJ, 130], F32R, tag="v")
                nc.gpsimd.memset(v_sb[:, :, 64:65].bitcast(F32), 1.0)
                nc.gpsimd.memset(v_sb[:, :, 129:130].bitcast(F32), 1.0)
                ctxs = ctxsp.tile([128, S], F32R, tag="ctxs")

                # ---- QKV projections ----
                for sw in range(NSW):
                    xcs = []
                    for dc in range(NDC):
                        xc = xcp.tile([128, SW], F32R, tag="xc")
                        nc.sync.dma_start(
                            xc[:],
                            xT_d.ap()[b, dc * 128:(dc + 1) * 128,
                                      sw * SW:(sw + 1) * SW])
                        xcs.append(xc)
                    for p in range(3):
                        ps = gp_p.tile([128, 512], F32, tag="gp")
                        with tc.tile_critical():
                            for i, dc in enumerate(reversed(range(NDC))):
                                nc.tensor.matmul(
                                    ps[:],
                                    wqkv_sb[:, dc, p * 128:(p + 1) * 128],
                                    xcs[dc][:],
                                    start=(i == 0), stop=(i == NDC - 1))
                        if p == 0:
                            dst = qT[:, sw * SW:(sw + 1) * SW]
                        elif p == 1:
                            dst = kT[:, sw * SW:(sw + 1) * SW]
                        else:
                            vts = vtp.tile([128, SW], F32, tag="vts")
                            dst = vts[:]
                        nc.vector.tensor_scalar_add(
                            dst, ps[:], bias_sb[:, p:p + 1])
                        if p == 2:
                            for t in range(SW // 128):
                                sj = sw * (SW // 128) + t
                                tp = gp_p.tile([128, 128], F32, tag="gp")
                                nc.tensor.transpose(
                                    tp[:], vts[:, t * 128:(t + 1) * 128],
                                    ident[:])
                                nc.vector.tensor_copy(v_sb[:, sj, 0:64],
                                                      tp[:, 0:64])
                                nc.vector.tensor_copy(v_sb[:, sj, 65:129],
                                                      tp[:, 64:128])

                # ---- attention ----
                for sw in range(NSW):
                    si_sl = slice(sw * SW, (sw + 1) * SW)
                    cc = [ctx_p.tile([65, 512], F32, tag=f"cc{_h}",
                                     name=f"cc{_h}", bufs=1)
                          for _h in range(HLOC)]
                    HALF = NSJ // 2
                    for half in range(2):
                        e_half = []
                        for sjh in range(HALF):
                            sj = half * HALF + sjh
                            sj_sl = slice(sj * 128, (sj + 1) * 128)
                            # both heads' scores into one 2-bank psum
                            # region, one fused exp over [128, 1024]
                            s_ps = sc_p.tile([128, 2, 512], F32, tag="sc")
                            for h in range(HLOC):
                                hp = slice(h * 64, (h + 1) * 64)
                                nc.tensor.matmul(s_ps[:, h, :],
                                                 kT[hp, sj_sl],
                                                 qT[hp, si_sl],
                                                 start=True, stop=True)
                            e = ep.tile([128, 2, SW], F32R, tag="e")
                            nc.scalar.activation(e[:], s_ps[:], AF.Exp,
                                                 scale=0.125)
                            e_half.append((sj, e))
                        # contiguous (atomic) per-head ctx chains; reversed
                        # so the chain starts only once every exp is done
                        for h in range(HLOC):
                            with tc.tile_critical():
                                for i, (sj, e) in enumerate(
                                        reversed(e_half)):
                                    nc.tensor.matmul(
                                        cc[h][:],
                                        v_sb[:, sj, h * 65:(h + 1) * 65],
                                        e[:, h, :],
                                        start=(half == 0 and i == 0),
                                        stop=(half == 1 and i == HALF - 1))
                    # evacuate ctx psum promptly (incl. denominator row 64),
                    # then normalize out of SBUF
                    for h in range(HLOC):
                        cs = smp.tile([65, 512], F32, tag=f"ccsb{h}",
                                      name=f"ccsb{h}")
                        nc.vector.tensor_copy(cs[:], cc[h][:])
                        rcp = smp.tile([1, 512], F32, tag=f"rcp{h}",
                                       name=f"rcp{h}")
                        nc.vector.reciprocal(rcp[:], cs[64:65, :])
                        rcr = smp.tile([1, 512], F32R, tag=f"rcr{h}",
                                       name=f"rcr{h}")
                        nc.vector.tensor_copy(rcr[:], rcp[:])
                        bc = gp_p.tile([64, 512], F32, tag="gp")
                        nc.tensor.matmul(bc[:], ones_sb[:], rcr[:],
                                         start=True, stop=True)
                        if h == 0:
                            nc.vector.tensor_mul(ctxs[0:64, si_sl],
                                                 cs[0:64, :], bc[:])
                        else:
                            c1t = smp.tile([64, 512], F32R, tag="c1t")
                            nc.vector.tensor_mul(c1t[:], cs[0:64, :], bc[:])
                            nc.sync.dma_start(ctxs[64:128, si_sl], c1t[:])
                    # ---- output projection for this window ----
                    for t in range(SW // 128):
                        si = sw * (SW // 128) + t
                        si_sl2 = slice(si * 128, (si + 1) * 128)
                        ost = osp.tile([128, D], F32, tag="ost")
                        for dhalf in range(2):
                            d_sl = slice(dhalf * 512, (dhalf + 1) * 512)
                            wpa = gp_p.tile([128, 512], F32, tag="gp")
                            wpb = gp_p.tile([128, 512], F32, tag="gp")
                            nc.tensor.matmul(wpa[:], ctxs[0:64, si_sl2],
                                             wo_sb[0:64, d_sl],
                                             start=True, stop=True)
                            nc.tensor.matmul(wpb[:], ctxs[64:128, si_sl2],
                                             wo_sb[64:128, d_sl],
                                             start=True, stop=True)
                            nc.vector.tensor_copy(ost[:, d_sl], wpa[:])
                            nc.vector.tensor_add(ost[:, d_sl],
                                                 ost[:, d_sl], wpb[:])
                        nc.sync.dma_start(out_d.ap()[b, si_sl2, :], ost[:])

        if loop_reps > 1:
            with tc.For_i(0, loop_reps, 1):
                _emit_body()
        else:
            for _rep in range(reps):
                _emit_body()

    nc.compile()
    return nc


_NC_CACHE: dict = {}


def _get_nc(reps: int = 1, loop_reps: int = 1):
    key = (reps, loop_reps)
    if key not in _NC_CACHE:
        _NC_CACHE[key] = _build(reps, loop_reps)
    return _NC_CACHE[key]


def _make_in_maps(x, Wq, bq, Wk, bk, Wv, bv, Wo, bo):
    xT = np.ascontiguousarray(np.transpose(x, (0, 2, 1)))  # [B, D, S]
    in_maps = []
    for core in range(NCORES):
        h0 = core * HLOC
        # [D, 128] per projection, heads side by side
        wq = np.concatenate([Wq[h0 + i] for i in range(HLOC)], axis=1)
        wk = np.concatenate([Wk[h0 + i] for i in range(HLOC)], axis=1)
        wv = np.concatenate([Wv[h0 + i] for i in range(HLOC)], axis=1)
        wqkv = np.ascontiguousarray(
            np.concatenate([wq, wk, wv], axis=1))  # [D, 384]
        bias = np.stack([
            np.concatenate([bq[h0 + i] for i in range(HLOC)]),
            np.concatenate([bk[h0 + i] for i in range(HLOC)]),
            np.concatenate([bv[h0 + i] for i in range(HLOC)]),
        ], axis=1).astype(np.float32)  # [128, 3]
        wo = np.ascontiguousarray(
            Wo[h0 * DH:(h0 + HLOC) * DH, :])  # [128, D]
        in_maps.append({
            "xT": xT,
            "wqkv": wqkv,
            "bqkv": bias,
            "wo": wo,
        })
    return in_maps


def kernel(x, Wq, bq, Wk, bk, Wv, bv, Wo, bo):
    x = np.asarray(x, dtype=np.float32)
    Wq = np.asarray(Wq, dtype=np.float32)
    bq = np.asarray(bq, dtype=np.float32)
    Wk = np.asarray(Wk, dtype=np.float32)
    bk = np.asarray(bk, dtype=np.float32)
    Wv = np.asarray(Wv, dtype=np.float32)
    bv = np.asarray(bv, dtype=np.float32)
    Wo = np.asarray(Wo, dtype=np.float32)
    bo = np.asarray(bo, dtype=np.float32)

    nc = _get_nc(reps=1)
    in_maps = _make_in_maps(x, Wq, bq, Wk, bk, Wv, bv, Wo, bo)
    res = bass_utils.run_bass_kernel_spmd(nc, in_maps, list(range(NCORES)))
    out = np.zeros((B, S, D), dtype=np.float32)
    for core in range(NCORES):
        out += res.results[core]["out"]
    out += bo[None, None, :]
    return out


class _TimedRunner:
    """Device-resident repeated executor for one prebuilt Bass module.

    Mirrors bass2jax.run_bass_via_pjrt's multi-core branch, but keeps
    inputs on device across calls and feeds each call's outputs back as
    the next call's donated output buffers (the kernel overwrites every
    output element, so initial contents don't matter)."""

    def __init__(self, nc, in_maps):
        import jax
        from jax.sharding import Mesh, PartitionSpec
        from jax.experimental.shard_map import shard_map
        from concourse import bass2jax, mybir as _mybir

        bass2jax.install_neuronx_cc_hook()
        n_cores = len(in_maps)
        partition_name = (nc.partition_id_tensor.name
                          if nc.partition_id_tensor else None)
        in_names, out_names, out_avals, zero_outs = [], [], [], []
        for alloc in nc.m.functions[0].allocations:
            if not isinstance(alloc, _mybir.MemoryLocationSet):
                continue
            name = alloc.memorylocations[0].name
            if alloc.kind == "ExternalInput":
                if name != partition_name:
                    in_names.append(name)
            elif alloc.kind == "ExternalOutput":
                out_names.append(name)
                shape = tuple(alloc.tensor_shape)
                dtype = _mybir.dt.np(alloc.dtype)
                out_avals.append(jax.core.ShapedArray(shape, dtype))
                zero_outs.append(np.zeros(shape, dtype))
        n_params = len(in_names)
        n_outs = len(out_avals)
        all_in_names = list(in_names) + list(out_names)
        if partition_name is not None:
            all_in_names.append(partition_name)
        donate = tuple(range(n_params, n_params + n_outs))

        def _body(*args):
            operands = list(args)
            if partition_name is not None:
                operands.append(bass2jax.partition_id_tensor())
            outs = bass2jax._bass_exec_p.bind(
                *operands,
                out_avals=tuple(out_avals),
                in_names=tuple(all_in_names),
                out_names=tuple(out_names),
                lowering_input_output_aliases=(),
                sim_require_finite=True,
                sim_require_nnan=True,
                nc=nc,
            )
            return tuple(outs)

        devices = jax.devices()[:n_cores]
        mesh = Mesh(np.asarray(devices), ("core",))
        in_specs = (PartitionSpec("core"),) * (n_params + n_outs)
        out_specs = (PartitionSpec("core"),) * n_outs
        self._fn = jax.jit(
            shard_map(_body, mesh=mesh, in_specs=in_specs,
                      out_specs=out_specs, check_rep=False),
            donate_argnums=donate, keep_unused=True)
        concat_in = [
            np.concatenate([np.asarray(in_maps[c][nm]) for c in range(n_cores)],
                           axis=0)
            for nm in in_names]
        self._in_dev = [jax.device_put(a) for a in concat_in]
        self._outs = [
            np.zeros((n_cores * z.shape[0], *z.shape[1:]), z.dtype)
            for z in zero_outs]
        self._jax = jax
        self.n_cores = n_cores
        self.out_names = out_names
        self.out_avals = out_avals

    def run(self):
        outs = self._fn(*self._in_dev, *self._outs)
        self._outs = list(outs)
        return outs

    def block(self):
        for o in self._outs:
            self._jax.block_until_ready(o)

    def timeit(self, n_warm=2, n_iter=10):
        import time
        for _ in range(n_warm):
            self.run()
        self.block()
        samples = []
        for _ in range(n_iter):
            t0 = time.perf_counter()
            self.run()
            self.block()
            samples.append(time.perf_counter() - t0)
        return samples

    def results(self):
        """Fetch per-core output dicts (host transfer)."""
        self.block()
        res = []
        for c in range(self.n_cores):
            d = {}
            for i, nm in enumerate(self.out_names):
                a = np.asarray(self._outs[i])
                d[nm] = a.reshape(self.n_cores, *self.out_avals[i].shape)[c]
            res.append(d)
        return res


def benchmark(x, Wq, bq, Wk, bk, Wv, bv, Wo, bo, loops=(201, 601),
              n_iter: int = 8):
    """Estimate HW exec time of one kernel body with a hardware For_i loop
    around the body: (t[R_hi] - t[R_lo]) / (R_hi - R_lo), device-resident
    I/O so per-call overhead is pure dispatch and cancels in the diff."""
    in_maps = _make_in_maps(x, Wq, bq, Wk, bk, Wv, bv, Wo, bo)
    lo, hi = loops
    stats = {}
    for lr in (lo, hi):
        nc = _get_nc(reps=1, loop_reps=lr)
        r = _TimedRunner(nc, in_maps)
        samples = r.timeit(n_iter=n_iter)
        stats[lr] = (min(samples), float(np.median(samples)))
        del r
    body_ns = (stats[hi][1] - stats[lo][1]) / (hi - lo) * 1e9
    return body_ns, stats


# revision 29
# speedup vs baseline: 1.2592x; 1.2592x over previous
"""Trainium2 Bass kernel for DP MultiHeadAttention.

Problem: B=2, S=2048, D=1024, H=16, DH=64 (fp32).
  q/k/v = per-head projections of x; scores = q k^T / 8; probs = softmax;
  ctx = probs @ v; out = concat-heads(ctx) @ Wo + bo.

Sharding: heads (tensor parallel) across 8 cores, 2 heads/core.
Each core computes its 2 heads' q/k/v + attention + the partial output
projection (its 128 rows of Wo); host sums the 8 partials + bo
(the "all-reduce" of the output projection, done at gather time).

Device algorithm per core (transposed-softmax flash-style layout):
  - Host supplies x pre-transposed: xT [B, D, S].
  - qT/kT [128=2*DH, S] = Wqkv^T-style projections (PE, f32r), bias added
    per-partition on DVE.
  - v produced transposed then PE-transposed back to natural [S, 2*DH],
    stored per sj-chunk as [128, 16, 130] with a ones-column per head
    (the ones column makes the ctx matmul also produce the softmax
    denominator for free).
  - scoresT[sj, si] per head via row-packed K=64 matmul pairs.
  - exp on ScalarE (scale=1/8 folded in, no max-subtraction: scores are
    O(5) so exp is safe in fp32).
  - ctx^T accumulated over sj on PE; row 64 = denominator.
  - recip on DVE, broadcast across partitions via K=1 PE outer product,
    ctx scaled on DVE -> ctxs [64, S] per head (f32r).
  - output projection: two K=64 accumulating matmuls (one per head)
    per [128, 512] output tile.

All matmuls use float32r (1 cycle/row vs 4 for strict fp32; ~1.5e-4 rel
err measured on HW) with fp32 PSUM accumulation.
"""
import os
import sys

for _p in ("/opt/trn_rl_repo",):
    if _p not in sys.path:
        sys.path.insert(0, _p)

import numpy as np

import concourse.bass as bass
from concourse import bacc, mybir, masks
from concourse import tile as ctile
from concourse import bass_utils
from contextlib import ExitStack

B, S, D, H = 2, 2048, 1024, 16
DH = D // H  # 64
NCORES = 8
HLOC = H // NCORES  # 2
SW = 512            # si window
NSW = S // SW       # 4
NSJ = S // 128      # 16
NDC = D // 128      # 8

F32 = mybir.dt.float32
F32R = mybir.dt.float32r
AF = mybir.ActivationFunctionType


def _build(reps: int = 1, loop_reps: int = 1):
    # The neuron NEFF cache keys can collide across different BIR contents
    # (observed during development); never trust a stale cache.
    import shutil
    shutil.rmtree(os.path.expanduser("~/.neuron-compile-cache"),
                  ignore_errors=True)
    nc = bacc.Bacc("TRN2", target_bir_lowering=False, debug=False,
                   num_devices=NCORES)
    xT_d = nc.dram_tensor("xT", [B, D, S], F32R, kind="ExternalInput")
    wqkv_d = nc.dram_tensor("wqkv", [D, 3 * 128], F32R, kind="ExternalInput")
    bias_d = nc.dram_tensor("bqkv", [128, 3], F32, kind="ExternalInput")
    wo_d = nc.dram_tensor("wo", [128, D], F32R, kind="ExternalInput")
    out_d = nc.dram_tensor("out", [B, S, D], F32, kind="ExternalOutput")

    with ctile.TileContext(nc) as tc, ExitStack() as ctx:
        const = ctx.enter_context(tc.tile_pool(name="const", bufs=1))
        xcp = ctx.enter_context(tc.tile_pool(name="xcp", bufs=10))
        qkp = ctx.enter_context(tc.tile_pool(name="qkp", bufs=2))
        vtp = ctx.enter_context(tc.tile_pool(name="vtp", bufs=2))
        ep = ctx.enter_context(tc.tile_pool(name="ep", bufs=10))
        ctxsp = ctx.enter_context(tc.tile_pool(name="ctxsp", bufs=2))
        smp = ctx.enter_context(tc.tile_pool(name="smp", bufs=2))
        osp = ctx.enter_context(tc.tile_pool(name="osp", bufs=3))
        sc_p = ctx.enter_context(
            tc.tile_pool(name="sc_p", bufs=2, space=bass.MemorySpace.PSUM))
        ctx_p = ctx.enter_context(
            tc.tile_pool(name="ctx_p", bufs=2, space=bass.MemorySpace.PSUM))
        gp_p = ctx.enter_context(
            tc.tile_pool(name="gp_p", bufs=2, space=bass.MemorySpace.PSUM))

        # ---- constants ----
        wqkv_sb = const.tile([128, NDC, 3 * 128], F32R)
        for dc in range(NDC):
            nc.sync.dma_start(wqkv_sb[:, dc, :],
                                wqkv_d.ap()[dc * 128:(dc + 1) * 128, :])
        bias_sb = const.tile([128, 3], F32)
        nc.sync.dma_start(bias_sb[:], bias_d.ap()[:])
        wo_sb = const.tile([128, D], F32R)
        nc.sync.dma_start(wo_sb[:], wo_d.ap()[:])
        ident = const.tile([128, 128], F32)
        masks.make_identity(nc, ident[:])
        # ones row (bcast matmul lhsT)
        ones_sb = const.tile([1, 64], F32R)
        nc.gpsimd.memset(ones_sb[:].bitcast(F32), 1.0)

        def _emit_body():
            for b in range(B):
                qT = qkp.tile([128, S], F32R, tag="qT")
                kT = qkp.tile([128, S], F32R, tag="kT")
                v_sb = vtp.tile([128, NSJ, 130], F32R, tag="v")
                nc.gpsimd.memset(v_sb[:, :, 64:65].bitcast(F32), 1.0)
                nc.gpsimd.memset(v_sb[:, :, 129:130].bitcast(F32), 1.0)
                ctxs = ctxsp.tile([128, S], F32R, tag="ctxs")

                # ---- QKV projections ----
                for sw in range(NSW):
                    xcs = []
                    for dc in range(NDC):
                        xc = xcp.tile([128, SW], F32R, tag="xc")
                        nc.sync.dma_start(
                            xc[:],
                            xT_d.ap()[b, dc * 128:(dc + 1) * 128,
                                      sw * SW:(sw + 1) * SW])
                        xcs.append(xc)
                    for p in range(3):
                        ps = gp_p.tile([128, 512], F32, tag="gp")
                        for i, dc in enumerate(reversed(range(NDC))):
                            nc.tensor.matmul(
                                ps[:],
                                wqkv_sb[:, dc, p * 128:(p + 1) * 128],
                                xcs[dc][:],
                                start=(i == 0), stop=(i == NDC - 1))
                        if p == 0:
                            dst = qT[:, sw * SW:(sw + 1) * SW]
                        elif p == 1:
                            dst = kT[:, sw * SW:(sw + 1) * SW]
                        else:
                            vts = vtp.tile([128, SW], F32, tag="vts")
                            dst = vts[:]
                        nc.vector.tensor_scalar_add(
                            dst, ps[:], bias_sb[:, p:p + 1])
                        if p == 2:
                            for t in range(SW // 128):
                                sj = sw * (SW // 128) + t
                                tp = gp_p.tile([128, 128], F32, tag="gp")
                                nc.tensor.transpose(
                                    tp[:], vts[:, t * 128:(t + 1) * 128],
                                    ident[:])
                                nc.vector.tensor_copy(v_sb[:, sj, 0:64],
                                                      tp[:, 0:64])
                                nc.vector.tensor_copy(v_sb[:, sj, 65:129],
                                                      tp[:, 64:128])

                # ---- attention ----
                for sw in range(NSW):
                    si_sl = slice(sw * SW, (sw + 1) * SW)
                    cc = [ctx_p.tile([65, 512], F32, tag=f"cc{_h}",
                                     name=f"cc{_h}", bufs=1)
                          for _h in range(HLOC)]
                    HALF = NSJ // 2
                    for half in range(2):
                        e_half = []
                        for sjh in range(HALF):
                            sj = half * HALF + sjh
                            sj_sl = slice(sj * 128, (sj + 1) * 128)
                            # both heads' scores into one 2-bank psum
                            # region, one fused exp over [128, 1024]
                            s_ps = sc_p.tile([128, 2, 512], F32, tag="sc")
                            for h in range(HLOC):
                                hp = slice(h * 64, (h + 1) * 64)
                                nc.tensor.matmul(s_ps[:, h, :],
                                                 kT[hp, sj_sl],
                                                 qT[hp, si_sl],
                                                 start=True, stop=True)
                            e = ep.tile([128, 2, SW], F32R, tag="e")
                            nc.scalar.activation(e[:], s_ps[:], AF.Exp,
                                                 scale=0.125)
                            e_half.append((sj, e))
                        # contiguous (atomic) per-head ctx chains; reversed
                        # so the chain starts only once every exp is done
                        for h in range(HLOC):
                            for i, (sj, e) in enumerate(reversed(e_half)):
                                nc.tensor.matmul(
                                    cc[h][:],
                                    v_sb[:, sj, h * 65:(h + 1) * 65],
                                    e[:, h, :],
                                    start=(half == 0 and i == 0),
                                    stop=(half == 1 and i == HALF - 1))
                    # evacuate ctx psum promptly (incl. denominator row 64),
                    # then normalize out of SBUF
                    for h in range(HLOC):
                        cs = smp.tile([65, 512], F32, tag=f"ccsb{h}",
                                      name=f"ccsb{h}")
                        nc.vector.tensor_copy(cs[:], cc[h][:])
                        rcp = smp.tile([1, 512], F32, tag=f"rcp{h}",
                                       name=f"rcp{h}")
                        nc.vector.reciprocal(rcp[:], cs[64:65, :])
                        rcr = smp.tile([1, 512], F32R, tag=f"rcr{h}",
                                       name=f"rcr{h}")
                        nc.vector.tensor_copy(rcr[:], rcp[:])
                        bc = gp_p.tile([64, 512], F32, tag="gp")
                        nc.tensor.matmul(bc[:], ones_sb[:], rcr[:],
                                         start=True, stop=True)
                        if h == 0:
                            nc.vector.tensor_mul(ctxs[0:64, si_sl],
                                                 cs[0:64, :], bc[:])
                        else:
                            c1t = smp.tile([64, 512], F32R, tag="c1t")
                            nc.vector.tensor_mul(c1t[:], cs[0:64, :], bc[:])
                            nc.sync.dma_start(ctxs[64:128, si_sl], c1t[:])
                    # ---- output projection for this window ----
                    for t in range(SW // 128):
                        si = sw * (SW // 128) + t
                        si_sl2 = slice(si * 128, (si + 1) * 128)
                        ost = osp.tile([128, D], F32, tag="ost")
                        for dhalf in range(2):
                            d_sl = slice(dhalf * 512, (dhalf + 1) * 512)
                            wpa = gp_p.tile([128, 512], F32, tag="gp")
                            wpb = gp_p.tile([128, 512], F32, tag="gp")
                            nc.tensor.matmul(wpa[:], ctxs[0:64, si_sl2],
                                             wo_sb[0:64, d_sl],
                                             start=True, stop=True)
                            nc.tensor.matmul(wpb[:], ctxs[64:128, si_sl2],
                                             wo_sb[64:128, d_sl],
                                             start=True, stop=True)
                            nc.vector.tensor_copy(ost[:, d_sl], wpa[:])
                            nc.vector.tensor_add(ost[:, d_sl],
                                                 ost[:, d_sl], wpb[:])
                        nc.sync.dma_start(out_d.ap()[b, si_sl2, :], ost[:])

        if loop_reps > 1:
            with tc.For_i(0, loop_reps, 1):
                _emit_body()
        else:
            for _rep in range(reps):
                _emit_body()

    nc.compile()
    return nc


_NC_CACHE: dict = {}


def _get_nc(reps: int = 1, loop_reps: int = 1):
    key = (reps, loop_reps)
    if key not in _NC_CACHE:
        _NC_CACHE[key] = _build(reps, loop_reps)
    return _NC_CACHE[key]


def _make_in_maps(x, Wq, bq, Wk, bk, Wv, bv, Wo, bo):
    xT = np.ascontiguousarray(np.transpose(x, (0, 2, 1)))  # [B, D, S]
    in_maps = []
    for core in range(NCORES):
        h0 = core * HLOC
        # [D, 128] per projection, heads side by side
        wq = np.concatenate([Wq[h0 + i] for i in range(HLOC)], axis=1)
        wk = np.concatenate([Wk[h0 + i] for i in range(HLOC)], axis=1)
        wv = np.concatenate([Wv[h0 + i] for i in range(HLOC)], axis=1)
        wqkv = np.ascontiguousarray(
            np.concatenate([wq, wk, wv], axis=1))  # [D, 384]
        bias = np.stack([
            np.concatenate([bq[h0 + i] for i in range(HLOC)]),
            np.concatenate([bk[h0 + i] for i in range(HLOC)]),
            np.concatenate([bv[h0 + i] for i in range(HLOC)]),
        ], axis=1).astype(np.float32)  # [128, 3]
        wo = np.ascontiguousarray(
            Wo[h0 * DH:(h0 + HLOC) * DH, :])  # [128, D]
        in_maps.append({
            "xT": xT,
            "wqkv": wqkv,
            "bqkv": bias,
            "wo": wo,
        })
    return in_maps


def kernel(x, Wq, bq, Wk, bk, Wv, bv, Wo, bo):
    x = np.asarray(x, dtype=np.float32)
    Wq = np.asarray(Wq, dtype=np.float32)
    bq = np.asarray(bq, dtype=np.float32)
    Wk = np.asarray(Wk, dtype=np.float32)
    bk = np.asarray(bk, dtype=np.float32)
    Wv = np.asarray(Wv, dtype=np.float32)
    bv = np.asarray(bv, dtype=np.float32)
    Wo = np.asarray(Wo, dtype=np.float32)
    bo = np.asarray(bo, dtype=np.float32)

    nc = _get_nc(reps=1)
    in_maps = _make_in_maps(x, Wq, bq, Wk, bk, Wv, bv, Wo, bo)
    res = bass_utils.run_bass_kernel_spmd(nc, in_maps, list(range(NCORES)))
    out = np.zeros((B, S, D), dtype=np.float32)
    for core in range(NCORES):
        out += res.results[core]["out"]
    out += bo[None, None, :]
    return out


class _TimedRunner:
    """Device-resident repeated executor for one prebuilt Bass module.

    Mirrors bass2jax.run_bass_via_pjrt's multi-core branch, but keeps
    inputs on device across calls and feeds each call's outputs back as
    the next call's donated output buffers (the kernel overwrites every
    output element, so initial contents don't matter)."""

    def __init__(self, nc, in_maps):
        import jax
        from jax.sharding import Mesh, PartitionSpec
        from jax.experimental.shard_map import shard_map
        from concourse import bass2jax, mybir as _mybir

        bass2jax.install_neuronx_cc_hook()
        n_cores = len(in_maps)
        partition_name = (nc.partition_id_tensor.name
                          if nc.partition_id_tensor else None)
        in_names, out_names, out_avals, zero_outs = [], [], [], []
        for alloc in nc.m.functions[0].allocations:
            if not isinstance(alloc, _mybir.MemoryLocationSet):
                continue
            name = alloc.memorylocations[0].name
            if alloc.kind == "ExternalInput":
                if name != partition_name:
                    in_names.append(name)
            elif alloc.kind == "ExternalOutput":
                out_names.append(name)
                shape = tuple(alloc.tensor_shape)
                dtype = _mybir.dt.np(alloc.dtype)
                out_avals.append(jax.core.ShapedArray(shape, dtype))
                zero_outs.append(np.zeros(shape, dtype))
        n_params = len(in_names)
        n_outs = len(out_avals)
        all_in_names = list(in_names) + list(out_names)
        if partition_name is not None:
            all_in_names.append(partition_name)
        donate = tuple(range(n_params, n_params + n_outs))

        def _body(*args):
            operands = list(args)
            if partition_name is not None:
                operands.append(bass2jax.partition_id_tensor())
            outs = bass2jax._bass_exec_p.bind(
                *operands,
                out_avals=tuple(out_avals),
                in_names=tuple(all_in_names),
                out_names=tuple(out_names),
                lowering_input_output_aliases=(),
                sim_require_finite=True,
                sim_require_nnan=True,
                nc=nc,
            )
            return tuple(outs)

        devices = jax.devices()[:n_cores]
        mesh = Mesh(np.asarray(devices), ("core",))
        in_specs = (PartitionSpec("core"),) * (n_params + n_outs)
        out_specs = (PartitionSpec("core"),) * n_outs
        self._fn = jax.jit(
            shard_map(_body, mesh=mesh, in_specs=in_specs,
                      out_specs=out_specs, check_rep=False),
            donate_argnums=donate, keep_unused=True)
        concat_in = [
            np.concatenate([np.asarray(in_maps[c][nm]) for c in range(n_cores)],
                           axis=0)
            for nm in in_names]
        self._in_dev = [jax.device_put(a) for a in concat_in]
        self._outs = [
            np.zeros((n_cores * z.shape[0], *z.shape[1:]), z.dtype)
            for z in zero_outs]
        self._jax = jax
        self.n_cores = n_cores
        self.out_names = out_names
        self.out_avals = out_avals

    def run(self):
        outs = self._fn(*self._in_dev, *self._outs)
        self._outs = list(outs)
        return outs

    def block(self):
        for o in self._outs:
            self._jax.block_until_ready(o)

    def timeit(self, n_warm=2, n_iter=10):
        import time
        for _ in range(n_warm):
            self.run()
        self.block()
        samples = []
        for _ in range(n_iter):
            t0 = time.perf_counter()
            self.run()
            self.block()
            samples.append(time.perf_counter() - t0)
        return samples

    def results(self):
        """Fetch per-core output dicts (host transfer)."""
        self.block()
        res = []
        for c in range(self.n_cores):
            d = {}
            for i, nm in enumerate(self.out_names):
                a = np.asarray(self._outs[i])
                d[nm] = a.reshape(self.n_cores, *self.out_avals[i].shape)[c]
            res.append(d)
        return res


def benchmark(x, Wq, bq, Wk, bk, Wv, bv, Wo, bo, loops=(201, 601),
              n_iter: int = 8):
    """Estimate HW exec time of one kernel body with a hardware For_i loop
    around the body: (t[R_hi] - t[R_lo]) / (R_hi - R_lo), device-resident
    I/O so per-call overhead is pure dispatch and cancels in the diff."""
    in_maps = _make_in_maps(x, Wq, bq, Wk, bk, Wv, bv, Wo, bo)
    lo, hi = loops
    stats = {}
    for lr in (lo, hi):
        nc = _get_nc(reps=1, loop_reps=lr)
        r = _TimedRunner(nc, in_maps)
        samples = r.timeit(n_iter=n_iter)
        stats[lr] = (min(samples), float(np.median(samples)))
        del r
    body_ns = (stats[hi][1] - stats[lo][1]) / (hi - lo) * 1e9
    return body_ns, stats


# revision 31
# speedup vs baseline: 1.3711x; 1.0889x over previous
"""Trainium2 Bass kernel for DP MultiHeadAttention.

Problem: B=2, S=2048, D=1024, H=16, DH=64 (fp32).
  q/k/v = per-head projections of x; scores = q k^T / 8; probs = softmax;
  ctx = probs @ v; out = concat-heads(ctx) @ Wo + bo.

Sharding: heads (tensor parallel) across 8 cores, 2 heads/core.
Each core computes its 2 heads' q/k/v + attention + the partial output
projection (its 128 rows of Wo); host sums the 8 partials + bo
(the "all-reduce" of the output projection, done at gather time).

Device algorithm per core (transposed-softmax flash-style layout):
  - Host supplies x pre-transposed: xT [B, D, S].
  - qT/kT [128=2*DH, S] = Wqkv^T-style projections (PE, f32r), bias added
    per-partition on DVE.
  - v produced transposed then PE-transposed back to natural [S, 2*DH],
    stored per sj-chunk as [128, 16, 130] with a ones-column per head
    (the ones column makes the ctx matmul also produce the softmax
    denominator for free).
  - scoresT[sj, si] per head via row-packed K=64 matmul pairs.
  - exp on ScalarE (scale=1/8 folded in, no max-subtraction: scores are
    O(5) so exp is safe in fp32).
  - ctx^T accumulated over sj on PE; row 64 = denominator.
  - recip on DVE, broadcast across partitions via K=1 PE outer product,
    ctx scaled on DVE -> ctxs [64, S] per head (f32r).
  - output projection: two K=64 accumulating matmuls (one per head)
    per [128, 512] output tile.

All matmuls use float32r (1 cycle/row vs 4 for strict fp32; ~1.5e-4 rel
err measured on HW) with fp32 PSUM accumulation.
"""
import os
import sys

for _p in ("/opt/trn_rl_repo",):
    if _p not in sys.path:
        sys.path.insert(0, _p)

import numpy as np

import concourse.bass as bass
from concourse import bacc, mybir, masks
from concourse import tile as ctile
from concourse import bass_utils
from contextlib import ExitStack

B, S, D, H = 2, 2048, 1024, 16
DH = D // H  # 64
NCORES = 8
HLOC = H // NCORES  # 2
SW = 512            # si window
NSW = S // SW       # 4
NSJ = S // 128      # 16
NDC = D // 128      # 8

F32 = mybir.dt.float32
F32R = mybir.dt.float32r
AF = mybir.ActivationFunctionType


def _build(reps: int = 1, loop_reps: int = 1):
    # The neuron NEFF cache keys can collide across different BIR contents
    # (observed during development); never trust a stale cache.
    import shutil
    shutil.rmtree(os.path.expanduser("~/.neuron-compile-cache"),
                  ignore_errors=True)
    nc = bacc.Bacc("TRN2", target_bir_lowering=False, debug=False,
                   num_devices=NCORES)
    xT_d = nc.dram_tensor("xT", [B, D, S], F32R, kind="ExternalInput")
    wqkv_d = nc.dram_tensor("wqkv", [D, 3 * 128], F32R, kind="ExternalInput")
    bias_d = nc.dram_tensor("bqkv", [128, 3], F32, kind="ExternalInput")
    wo_d = nc.dram_tensor("wo", [128, D], F32R, kind="ExternalInput")
    out_d = nc.dram_tensor("out", [B, S, D], F32, kind="ExternalOutput")

    with ctile.TileContext(nc) as tc, ExitStack() as ctx:
        const = ctx.enter_context(tc.tile_pool(name="const", bufs=1))
        xcp = ctx.enter_context(tc.tile_pool(name="xcp", bufs=16))
        qkp = ctx.enter_context(tc.tile_pool(name="qkp", bufs=2))
        vtp = ctx.enter_context(tc.tile_pool(name="vtp", bufs=2))
        ep = ctx.enter_context(tc.tile_pool(name="ep", bufs=6))
        ctxsp = ctx.enter_context(tc.tile_pool(name="ctxsp", bufs=2))
        smp = ctx.enter_context(tc.tile_pool(name="smp", bufs=2))
        osp = ctx.enter_context(tc.tile_pool(name="osp", bufs=3))
        sc_p = ctx.enter_context(
            tc.tile_pool(name="sc_p", bufs=2, space=bass.MemorySpace.PSUM))
        ctx_p = ctx.enter_context(
            tc.tile_pool(name="ctx_p", bufs=2, space=bass.MemorySpace.PSUM))
        gp_p = ctx.enter_context(
            tc.tile_pool(name="gp_p", bufs=2, space=bass.MemorySpace.PSUM))

        # ---- constants ----
        wqkv_sb = const.tile([128, NDC, 3 * 128], F32R)
        for dc in range(NDC):
            nc.sync.dma_start(wqkv_sb[:, dc, :],
                                wqkv_d.ap()[dc * 128:(dc + 1) * 128, :])
        bias_sb = const.tile([128, 3], F32)
        nc.sync.dma_start(bias_sb[:], bias_d.ap()[:])
        wo_sb = const.tile([128, D], F32R)
        nc.sync.dma_start(wo_sb[:], wo_d.ap()[:])
        ident = const.tile([128, 128], F32)
        masks.make_identity(nc, ident[:])
        # ones row (bcast matmul lhsT)
        ones_sb = const.tile([1, 64], F32R)
        nc.gpsimd.memset(ones_sb[:].bitcast(F32), 1.0)

        def _emit_body():
            for b in range(B):
                qT = qkp.tile([128, S], F32R, tag="qT")
                kT = qkp.tile([128, S], F32R, tag="kT")
                v_sb = vtp.tile([128, NSJ, 130], F32R, tag="v")
                nc.gpsimd.memset(v_sb[:, :, 64:65].bitcast(F32), 1.0)
                nc.gpsimd.memset(v_sb[:, :, 129:130].bitcast(F32), 1.0)
                ctxs = ctxsp.tile([128, S], F32R, tag="ctxs")

                # ---- QKV projections ----
                vts_all = []
                for sw in range(NSW):
                    xcs = []
                    for dc in range(NDC):
                        xc = xcp.tile([128, SW], F32R, tag="xc")
                        nc.sync.dma_start(
                            xc[:],
                            xT_d.ap()[b, dc * 128:(dc + 1) * 128,
                                      sw * SW:(sw + 1) * SW])
                        xcs.append(xc)
                    for p in range(3):
                        ps = gp_p.tile([128, 512], F32, tag="gp")
                        # reversed: first link waits the LAST x-chunk DMA, so
                        # the chain enters only when fully ready and runs
                        # dense (chain re-entry after another mm costs ~500ns)
                        for i, dc in enumerate(reversed(range(NDC))):
                            nc.tensor.matmul(
                                ps[:],
                                wqkv_sb[:, dc, p * 128:(p + 1) * 128],
                                xcs[dc][:],
                                start=(i == 0), stop=(i == NDC - 1))
                        if p == 0:
                            dst = qT[:, sw * SW:(sw + 1) * SW]
                        elif p == 1:
                            dst = kT[:, sw * SW:(sw + 1) * SW]
                        else:
                            vts = vtp.tile([128, SW], F32, tag="vts",
                                           bufs=NSW)
                            vts_all.append(vts)
                            dst = vts[:]
                        nc.vector.tensor_scalar_add(
                            dst, ps[:], bias_sb[:, p:p + 1])
                # batched v-transposes (non-acc singles, out of the chains)
                for sw in range(NSW):
                    vts = vts_all[sw]
                    for t in range(SW // 128):
                        sj = sw * (SW // 128) + t
                        tp = gp_p.tile([128, 128], F32, tag="gp")
                        nc.tensor.transpose(
                            tp[:], vts[:, t * 128:(t + 1) * 128], ident[:])
                        nc.vector.tensor_copy(v_sb[:, sj, 0:64],
                                              tp[:, 0:64])
                        nc.vector.tensor_copy(v_sb[:, sj, 65:129],
                                              tp[:, 64:128])

                # ---- attention ----
                for sw in range(NSW):
                    si_sl = slice(sw * SW, (sw + 1) * SW)
                    cc = [ctx_p.tile([65, 512], F32, tag=f"cc{_h}",
                                     name=f"cc{_h}", bufs=1)
                          for _h in range(HLOC)]
                    for sj in range(NSJ):
                        sj_sl = slice(sj * 128, (sj + 1) * 128)
                        # both heads' scores into one 2-bank psum region,
                        # one fused exp over [128, 1024]
                        s_ps = sc_p.tile([128, 2, 512], F32, tag="sc")
                        for h in range(HLOC):
                            hp = slice(h * 64, (h + 1) * 64)
                            nc.tensor.matmul(s_ps[:, h, :], kT[hp, sj_sl],
                                             qT[hp, si_sl],
                                             start=True, stop=True)
                        e = ep.tile([128, 2, SW], F32R, tag="e")
                        nc.scalar.activation(e[:], s_ps[:], AF.Exp,
                                             scale=0.125)
                        st, sp = (sj == 0), (sj == NSJ - 1)
                        for h in range(HLOC):
                            nc.tensor.matmul(
                                cc[h][:], v_sb[:, sj, h * 65:(h + 1) * 65],
                                e[:, h, :], start=st, stop=sp)
                    # evacuate ctx psum promptly (incl. denominator row 64),
                    # then normalize out of SBUF
                    for h in range(HLOC):
                        cs = smp.tile([65, 512], F32, tag=f"ccsb{h}",
                                      name=f"ccsb{h}")
                        nc.vector.tensor_copy(cs[:], cc[h][:])
                        rcp = smp.tile([1, 512], F32, tag=f"rcp{h}",
                                       name=f"rcp{h}")
                        nc.vector.reciprocal(rcp[:], cs[64:65, :])
                        rcr = smp.tile([1, 512], F32R, tag=f"rcr{h}",
                                       name=f"rcr{h}")
                        nc.vector.tensor_copy(rcr[:], rcp[:])
                        bc = gp_p.tile([64, 512], F32, tag="gp")
                        nc.tensor.matmul(bc[:], ones_sb[:], rcr[:],
                                         start=True, stop=True)
                        if h == 0:
                            nc.vector.tensor_mul(ctxs[0:64, si_sl],
                                                 cs[0:64, :], bc[:])
                        else:
                            c1t = smp.tile([64, 512], F32R, tag="c1t")
                            nc.vector.tensor_mul(c1t[:], cs[0:64, :], bc[:])
                            nc.sync.dma_start(ctxs[64:128, si_sl], c1t[:])
                    # ---- output projection for this window ----
                    for t in range(SW // 128):
                        si = sw * (SW // 128) + t
                        si_sl2 = slice(si * 128, (si + 1) * 128)
                        ost = osp.tile([128, D], F32, tag="ost")
                        for dhalf in range(2):
                            d_sl = slice(dhalf * 512, (dhalf + 1) * 512)
                            wpa = gp_p.tile([128, 512], F32, tag="gp")
                            wpb = gp_p.tile([128, 512], F32, tag="gp")
                            nc.tensor.matmul(wpa[:], ctxs[0:64, si_sl2],
                                             wo_sb[0:64, d_sl],
                                             start=True, stop=True)
                            nc.tensor.matmul(wpb[:], ctxs[64:128, si_sl2],
                                             wo_sb[64:128, d_sl],
                                             start=True, stop=True)
                            nc.vector.tensor_copy(ost[:, d_sl], wpa[:])
                            nc.vector.tensor_add(ost[:, d_sl],
                                                 ost[:, d_sl], wpb[:])
                        nc.sync.dma_start(out_d.ap()[b, si_sl2, :], ost[:])

        if loop_reps > 1:
            with tc.For_i(0, loop_reps, 1):
                _emit_body()
        else:
            for _rep in range(reps):
                _emit_body()

    nc.compile()
    return nc


_NC_CACHE: dict = {}


def _get_nc(reps: int = 1, loop_reps: int = 1):
    key = (reps, loop_reps)
    if key not in _NC_CACHE:
        _NC_CACHE[key] = _build(reps, loop_reps)
    return _NC_CACHE[key]


def _make_in_maps(x, Wq, bq, Wk, bk, Wv, bv, Wo, bo):
    xT = np.ascontiguousarray(np.transpose(x, (0, 2, 1)))  # [B, D, S]
    in_maps = []
    for core in range(NCORES):
        h0 = core * HLOC
        # [D, 128] per projection, heads side by side
        wq = np.concatenate([Wq[h0 + i] for i in range(HLOC)], axis=1)
        wk = np.concatenate([Wk[h0 + i] for i in range(HLOC)], axis=1)
        wv = np.concatenate([Wv[h0 + i] for i in range(HLOC)], axis=1)
        wqkv = np.ascontiguousarray(
            np.concatenate([wq, wk, wv], axis=1))  # [D, 384]
        bias = np.stack([
            np.concatenate([bq[h0 + i] for i in range(HLOC)]),
            np.concatenate([bk[h0 + i] for i in range(HLOC)]),
            np.concatenate([bv[h0 + i] for i in range(HLOC)]),
        ], axis=1).astype(np.float32)  # [128, 3]
        wo = np.ascontiguousarray(
            Wo[h0 * DH:(h0 + HLOC) * DH, :])  # [128, D]
        in_maps.append({
            "xT": xT,
            "wqkv": wqkv,
            "bqkv": bias,
            "wo": wo,
        })
    return in_maps


def kernel(x, Wq, bq, Wk, bk, Wv, bv, Wo, bo):
    x = np.asarray(x, dtype=np.float32)
    Wq = np.asarray(Wq, dtype=np.float32)
    bq = np.asarray(bq, dtype=np.float32)
    Wk = np.asarray(Wk, dtype=np.float32)
    bk = np.asarray(bk, dtype=np.float32)
    Wv = np.asarray(Wv, dtype=np.float32)
    bv = np.asarray(bv, dtype=np.float32)
    Wo = np.asarray(Wo, dtype=np.float32)
    bo = np.asarray(bo, dtype=np.float32)

    nc = _get_nc(reps=1)
    in_maps = _make_in_maps(x, Wq, bq, Wk, bk, Wv, bv, Wo, bo)
    res = bass_utils.run_bass_kernel_spmd(nc, in_maps, list(range(NCORES)))
    out = np.zeros((B, S, D), dtype=np.float32)
    for core in range(NCORES):
        out += res.results[core]["out"]
    out += bo[None, None, :]
    return out


class _TimedRunner:
    """Device-resident repeated executor for one prebuilt Bass module.

    Mirrors bass2jax.run_bass_via_pjrt's multi-core branch, but keeps
    inputs on device across calls and feeds each call's outputs back as
    the next call's donated output buffers (the kernel overwrites every
    output element, so initial contents don't matter)."""

    def __init__(self, nc, in_maps):
        import jax
        from jax.sharding import Mesh, PartitionSpec
        from jax.experimental.shard_map import shard_map
        from concourse import bass2jax, mybir as _mybir

        bass2jax.install_neuronx_cc_hook()
        n_cores = len(in_maps)
        partition_name = (nc.partition_id_tensor.name
                          if nc.partition_id_tensor else None)
        in_names, out_names, out_avals, zero_outs = [], [], [], []
        for alloc in nc.m.functions[0].allocations:
            if not isinstance(alloc, _mybir.MemoryLocationSet):
                continue
            name = alloc.memorylocations[0].name
            if alloc.kind == "ExternalInput":
                if name != partition_name:
                    in_names.append(name)
            elif alloc.kind == "ExternalOutput":
                out_names.append(name)
                shape = tuple(alloc.tensor_shape)
                dtype = _mybir.dt.np(alloc.dtype)
                out_avals.append(jax.core.ShapedArray(shape, dtype))
                zero_outs.append(np.zeros(shape, dtype))
        n_params = len(in_names)
        n_outs = len(out_avals)
        all_in_names = list(in_names) + list(out_names)
        if partition_name is not None:
            all_in_names.append(partition_name)
        donate = tuple(range(n_params, n_params + n_outs))

        def _body(*args):
            operands = list(args)
            if partition_name is not None:
                operands.append(bass2jax.partition_id_tensor())
            outs = bass2jax._bass_exec_p.bind(
                *operands,
                out_avals=tuple(out_avals),
                in_names=tuple(all_in_names),
                out_names=tuple(out_names),
                lowering_input_output_aliases=(),
                sim_require_finite=True,
                sim_require_nnan=True,
                nc=nc,
            )
            return tuple(outs)

        devices = jax.devices()[:n_cores]
        mesh = Mesh(np.asarray(devices), ("core",))
        in_specs = (PartitionSpec("core"),) * (n_params + n_outs)
        out_specs = (PartitionSpec("core"),) * n_outs
        self._fn = jax.jit(
            shard_map(_body, mesh=mesh, in_specs=in_specs,
                      out_specs=out_specs, check_rep=False),
            donate_argnums=donate, keep_unused=True)
        concat_in = [
            np.concatenate([np.asarray(in_maps[c][nm]) for c in range(n_cores)],
                           axis=0)
            for nm in in_names]
        self._in_dev = [jax.device_put(a) for a in concat_in]
        self._outs = [
            np.zeros((n_cores * z.shape[0], *z.shape[1:]), z.dtype)
            for z in zero_outs]
        self._jax = jax
        self.n_cores = n_cores
        self.out_names = out_names
        self.out_avals = out_avals

    def run(self):
        outs = self._fn(*self._in_dev, *self._outs)
        self._outs = list(outs)
        return outs

    def block(self):
        for o in self._outs:
            self._jax.block_until_ready(o)

    def timeit(self, n_warm=2, n_iter=10):
        import time
        for _ in range(n_warm):
            self.run()
        self.block()
        samples = []
        for _ in range(n_iter):
            t0 = time.perf_counter()
            self.run()
            self.block()
            samples.append(time.perf_counter() - t0)
        return samples

    def results(self):
        """Fetch per-core output dicts (host transfer)."""
        self.block()
        res = []
        for c in range(self.n_cores):
            d = {}
            for i, nm in enumerate(self.out_names):
                a = np.asarray(self._outs[i])
                d[nm] = a.reshape(self.n_cores, *self.out_avals[i].shape)[c]
            res.append(d)
        return res


def benchmark(x, Wq, bq, Wk, bk, Wv, bv, Wo, bo, loops=(201, 601),
              n_iter: int = 8):
    """Estimate HW exec time of one kernel body with a hardware For_i loop
    around the body: (t[R_hi] - t[R_lo]) / (R_hi - R_lo), device-resident
    I/O so per-call overhead is pure dispatch and cancels in the diff."""
    in_maps = _make_in_maps(x, Wq, bq, Wk, bk, Wv, bv, Wo, bo)
    lo, hi = loops
    stats = {}
    for lr in (lo, hi):
        nc = _get_nc(reps=1, loop_reps=lr)
        r = _TimedRunner(nc, in_maps)
        samples = r.timeit(n_iter=n_iter)
        stats[lr] = (min(samples), float(np.median(samples)))
        del r
    body_ns = (stats[hi][1] - stats[lo][1]) / (hi - lo) * 1e9
    return body_ns, stats
